# revision 30
# baseline (speedup 1.0000x reference)
"""Multi-head causal attention (B=4, T=2048, E=1024, H=16) on 8 NeuronCores.

Sharding: core = (batch b, head-group g of 8 heads). Each core computes its
heads' attention + a partial output projection; host sums the two partials
per batch and adds the bias (the "all-reduce" of the tensor-parallel plan).

Device layout (per core):
  xT  [E, T]   x[b] transposed on host; e on partitions (8 chunks of 128).
  wq/wk/wv [E, 512]: this core's 8 heads' weights, columns = h_local*64+d.
  wpT [512, E]: Wp rows for this core's 512 e-dims, transposed on host.
  QT/KT pair tiles [(2 heads' d)=128, T]; V tiles [t, 193] carry both heads'
  values plus ones/zeros columns so the PV matmuls emit softmax denominators
  into spare PSUM rows:
    cols 0:64    V_h0   -> ota rows 0:64   = OT_h0
    col  64      ones   -> ota row 64      = l_h0
    cols 65:97,98:129 zeros -> otb rows 0:31,33:63 = 0
    col  97      ones   -> otb row 32      = l_h1
    cols 129:193 V_h1   -> otb rows 64:128 = OT_h1 (lands on right partitions)
  Scores are computed transposed ST[t, s] (two heads' K=64 matmuls), exp on
  ACT with the 1/8 scale folded in, causality via reduced-width matmuls plus
  one [128,128] triangle mask on the diagonal subtile. Reciprocal runs on 2
  partitions (cost scales with free-size, not lanes), then a step-0 free-dim
  DMA replicates it across partitions for the normalize multiply. The
  normalized OT[(h,d), s] is exactly the lhsT the output projection needs.

FAST mode (default) tags matmul operands float32r (tf32-like, 11-bit
mantissa, 1 PE cycle/row); PRECISE mode keeps everything fp32 (4 cycles/row).
Set BASS_MHA_PRECISE=1 to force the exact variant.
"""

import os
import sys
from contextlib import ExitStack

import numpy as np

sys.path.insert(0, "/opt/trn_rl_repo")

import concourse.bass as bass
import concourse.tile as tile
from concourse import mybir
from concourse.bass_utils import run_bass_kernel_spmd

f32 = mybir.dt.float32
f32r = mybir.dt.float32r

B, T, E, H = 4, 2048, 1024, 16
DH = E // H          # 64
P = 128              # partitions
EC = E // P          # 8 e-chunks
NP = 4               # head pairs per core (8 heads)
SBW = 512            # s-block width
SB = T // SBW        # 4 s-blocks
TT = T // P          # 16 t-tiles
VW = 193             # V tile width: V0|1|zeros|1|V1
NCORES = 8

_PROGS = {}
LAST = {}


def _split_excess_waits(nc, limit=1):
    """walrus in this container encodes at most one sync-wait per
    instruction; move extras onto same-engine NOPs placed just before."""
    for fn in nc.m.functions:
        for bb in fn.blocks:
            out = []
            changed = False
            for inst in bb.instructions:
                si = inst.sync_info
                if si is not None and si.on_wait and len(si.on_wait) > limit:
                    waits = list(si.on_wait)
                    extra, keep = waits[:-limit], waits[-limit:]
                    for k, w in enumerate(extra):
                        nop = mybir.InstNoOp(
                            name=f"{inst.name}-wsplit{k}", ins=[], outs=[]
                        )
                        nop.engine = inst.engine
                        nop.sync_info = type(si)(on_wait=[w], on_update=[])
                        nc.register_instruction(nop)
                        out.append(nop)
                    si.on_wait = keep
                    changed = True
                out.append(inst)
            if changed:
                bb.instructions = out
    return nc


def _replicate_row_ap(src_row):
    """AP reading one SBUF partition row [1, W] as [1, 64, W] via a step-0
    free dim — DMA'd to a [64, W] destination this replicates the row."""
    return bass.AP(
        tensor=src_row.tensor,
        offset=src_row.offset,
        ap=[list(src_row.ap[0]), [0, 64], list(src_row.ap[1])],
    )


def _build_body(nc, tc, ctx, DT, xT_d, wq_d, wk_d, wv_d, wpT_d, tri_d, y_d):
    Exp = mybir.ActivationFunctionType.Exp
    Ln = mybir.ActivationFunctionType.Ln

    persist = ctx.enter_context(tc.tile_pool(name="persist", bufs=1))
    QT = [persist.tile([P, T], DT, tag=f"qt{p}", name=f"qt{p}") for p in range(NP)]
    V = [persist.tile([P, TT, VW], DT, tag=f"v{p}", name=f"v{p}") for p in range(NP)]
    tri_sb = persist.tile([P, P], DT, tag="tri")
    nc.sync.dma_start(out=tri_sb, in_=tri_d)

    # K staging in DRAM, already in the zero-padded KTA/KTB layout
    kta_dram = nc.dram_tensor("kta_dram", [NP, P, T], DT).ap()
    ktb_dram = nc.dram_tensor("ktb_dram", [NP, P, T], DT).ap()
    zero_sb = persist.tile([P, SBW], DT, tag="zeros")
    nc.vector.memset(zero_sb.bitcast(f32), 0.0)
    for p_ in range(NP):
        for mb in range(SB):
            mblk = slice(mb * SBW, (mb + 1) * SBW)
            nc.gpsimd.dma_start(
                out=kta_dram[p_][64:128, mblk], in_=zero_sb[0:64, :])
            nc.gpsimd.dma_start(
                out=ktb_dram[p_][0:64, mblk], in_=zero_sb[0:64, :])

    # ---------------- Phase 1: QKV projections ----------------
    with tc.tile_pool(name="xp", bufs=1) as xp:
        xTs = xp.tile([P, EC, T], DT)
        nc.sync.dma_start(out=xTs[:, 0, :], in_=xT_d[0:P, :])

        _ps1_ctx = ExitStack()
        ps1 = _ps1_ctx.enter_context(tc.tile_pool(name="ps1", bufs=4, space="PSUM"))

        wpool = ctx_w = ExitStack()
        wkp = ctx_w.enter_context(tc.tile_pool(name="wk", bufs=1))
        wqp = ctx_w.enter_context(tc.tile_pool(name="wq", bufs=1))
        wvp = ctx_w.enter_context(tc.tile_pool(name="wv", bufs=1))
        wks = wkp.tile([P, EC, NP * P], DT)
        wqs = wqp.tile([P, EC, NP * P], DT)
        wvs = wvp.tile([P, EC, NP * P], DT)
        wv_r = wv_d.rearrange("(c p) m -> p c m", p=P)
        wq_r = wq_d.rearrange("(c p) m -> p c m", p=P)
        wk_r = wk_d.rearrange("(c p) m -> p c m", p=P)
        # issue in matmul consumption order: (wv_c, xT_c) pairs, then wq, wk
        nc.sync.dma_start(out=wvs[:, 0, :], in_=wv_r[:, 0, :])
        for c in range(1, EC):
            nc.sync.dma_start(out=wvs[:, c, :], in_=wv_r[:, c, :])
            nc.sync.dma_start(out=xTs[:, c, :], in_=xT_d[c * P:(c + 1) * P, :])
        for c in range(EC):
            nc.sync.dma_start(out=wqs[:, c, :], in_=wq_r[:, c, :])
        for c in range(EC):
            nc.sync.dma_start(out=wks[:, c, :], in_=wk_r[:, c, :])

        # V natural for all 4 pairs at once (N=512), scattered into V tiles
        if True:
            for p_ in range(NP):
                nc.vector.memset(V[p_][:, :, 64:65].bitcast(f32), 1.0)
                nc.vector.memset(V[p_][:, :, 65:97].bitcast(f32), 0.0)
                nc.vector.memset(V[p_][:, :, 97:98].bitcast(f32), 1.0)
                nc.vector.memset(V[p_][:, :, 98:129].bitcast(f32), 0.0)
            for tt in range(TT):
                ps = ps1.tile([P, NP * P], f32, tag="ps1", name="ps")
                for c in range(EC):
                    nc.tensor.matmul(
                        ps,
                        lhsT=xTs[:, c, tt * P:(tt + 1) * P],
                        rhs=wvs[:, c, :],
                        start=(c == 0), stop=(c == EC - 1),
                    )
                for p_ in range(NP):
                    nc.vector.tensor_copy(
                        V[p_][:, tt, 0:64], ps[:, p_ * P:p_ * P + 64])
                    nc.vector.tensor_copy(
                        V[p_][:, tt, 129:193], ps[:, p_ * P + 64:(p_ + 1) * P])

        # QT pairs [(2 heads' d), s]
        if True:
            for p_ in range(NP):
                for m in range(SB):
                    ps = ps1.tile([P, SBW], f32, tag="ps1", name="ps")
                    for c in range(EC):
                        nc.tensor.matmul(
                            ps,
                            lhsT=wqs[:, c, p_ * P:(p_ + 1) * P],
                            rhs=xTs[:, c, m * SBW:(m + 1) * SBW],
                            start=(c == 0), stop=(c == EC - 1),
                        )
                    nc.vector.tensor_copy(QT[p_][:, m * SBW:(m + 1) * SBW], ps)

        # KT pairs [(2 heads' d), t] -> straight to DRAM staging via bounce
        with tc.tile_pool(name="bounce", bufs=3) as bncp:
            for p_ in range(NP):
                for m in range(SB):
                    ps = ps1.tile([P, SBW], f32, tag="ps1", name="ps")
                    for c in range(EC):
                        nc.tensor.matmul(
                            ps,
                            lhsT=wks[:, c, p_ * P:(p_ + 1) * P],
                            rhs=xTs[:, c, m * SBW:(m + 1) * SBW],
                            start=(c == 0), stop=(c == EC - 1),
                        )
                    bnc = bncp.tile([P, SBW], DT)
                    nc.vector.tensor_copy(bnc, ps)
                    mblk = slice(m * SBW, (m + 1) * SBW)
                    nc.gpsimd.dma_start(
                        out=kta_dram[p_][0:64, mblk], in_=bnc[0:64, :])
                    nc.gpsimd.dma_start(
                        out=ktb_dram[p_][64:128, mblk], in_=bnc[64:128, :])
        ctx_w.close()

    _ps1_ctx.close()

    # ---------------- Phase 2: causal attention ----------------
    # zero-padded K=128 score operands (rolling, rebuilt per pair):
    # KTA = [KT_h0; 0], KTB = [0; KT_h1]
    ktab_pool = ctx.enter_context(tc.tile_pool(name="ktab", bufs=2))

    otp = ctx.enter_context(tc.tile_pool(name="otp", bufs=1))
    OT = [otp.tile([P, T], DT, tag=f"ot{p}", name=f"ot{p}") for p in range(NP)]
    wpp = ctx.enter_context(tc.tile_pool(name="wp", bufs=1))
    wps = wpp.tile([P, NP, E], DT)
    for c in range(NP):
        nc.sync.dma_start(
            out=wps[:, c, :],
            in_=wpT_d.rearrange("(c p) m -> p c m", p=P)[:, c, :])
    with tc.tile_pool(name="pt", bufs=4) as ptp, \
         tc.tile_pool(name="lsb", bufs=2) as lsp, \
         tc.tile_pool(name="rsb", bufs=2) as rsp, \
         tc.tile_pool(name="rep", bufs=2) as repp, \
         tc.tile_pool(name="psst", bufs=2, space="PSUM") as psst, \
         tc.tile_pool(name="psota", bufs=2, space="PSUM") as psota, \
         tc.tile_pool(name="psotb", bufs=2, space="PSUM") as psotb:
        for p_ in range(NP):
            qt, vt, oc = QT[p_], V[p_], OT[p_]
            kta = ktab_pool.tile([P, T], DT, tag="kta", name="kta")
            ktb = ktab_pool.tile([P, T], DT, tag="ktb", name="ktb")
            nc.sync.dma_start(out=kta, in_=kta_dram[p_])
            nc.sync.dma_start(out=ktb, in_=ktb_dram[p_])
            for j in range(SB):
                ntt = 4 * (j + 1)
                ota = psota.tile([P, SBW], f32)
                otb = psotb.tile([P, SBW], f32)
                def _pv(pv_args):
                    pt_, s_lo_, i_ = pv_args
                    nc.tensor.matmul(
                        ota[:, s_lo_:SBW],
                        lhsT=vt[:, i_, 0:P],
                        rhs=pt_[:, s_lo_:SBW],
                        start=(i_ == 0), stop=(i_ == ntt - 1),
                    )
                    nc.tensor.matmul(
                        otb[:, s_lo_:SBW],
                        lhsT=vt[:, i_, 65:VW],
                        rhs=pt_[:, SBW + s_lo_:2 * SBW],
                        start=(i_ == 0), stop=(i_ == ntt - 1),
                    )

                pv_pending = None
                for i in range(ntt):
                    dd = i - 4 * j
                    s_lo = P * dd if dd >= 0 else 0
                    st = psst.tile([P, 2 * SBW], f32)
                    nc.tensor.matmul(
                        st[:, s_lo:SBW],
                        lhsT=kta[:, i * P:(i + 1) * P],
                        rhs=qt[:, j * SBW + s_lo:(j + 1) * SBW],
                        start=True, stop=True,
                    )
                    nc.tensor.matmul(
                        st[:, SBW + s_lo:2 * SBW],
                        lhsT=ktb[:, i * P:(i + 1) * P],
                        rhs=qt[:, j * SBW + s_lo:(j + 1) * SBW],
                        start=True, stop=True,
                    )
                    pt = ptp.tile([P, 2 * SBW], DT)
                    st3 = st.rearrange("p (h w) -> p h w", h=2)[:, :, s_lo:SBW]
                    pt3 = pt.rearrange("p (h w) -> p h w", h=2)[:, :, s_lo:SBW]
                    nc.scalar.activation(pt3, st3, Exp, bias=0.0, scale=0.125)
                    if dd >= 0:
                        nc.vector.tensor_mul(
                            pt[:, s_lo:s_lo + P], pt[:, s_lo:s_lo + P], tri_sb)
                        nc.vector.tensor_mul(
                            pt[:, SBW + s_lo:SBW + s_lo + P],
                            pt[:, SBW + s_lo:SBW + s_lo + P], tri_sb)
                    # software pipeline: PV for the previous iter issues after
                    # this iter's scores, so PE never waits on this iter's exp
                    if pv_pending is not None:
                        _pv(pv_pending)
                    pv_pending = (pt, s_lo, i)
                _pv(pv_pending)
                # finalize: l_h0 = ota row 64, l_h1 = otb row 32
                # deprioritized so it fills ACT/DVE idle slots instead of
                # bubbling the next block's score->exp->PV pipeline
                _pri0 = tc.cur_priority
                tc.cur_priority = _pri0 + 16
                r_sb = rsp.tile([P, SBW], f32)
                if DT is f32r:
                    # 1/l = exp(-ln(l)) on ACT (LUT err ~1e-5 << f32r rounding)
                    ln_sb = lsp.tile([P, SBW], f32)
                    nc.scalar.activation(ln_sb[64:65, :], ota[64:65, :], Ln)
                    nc.scalar.activation(ln_sb[32:33, :], otb[32:33, :], Ln)
                    nc.scalar.activation(r_sb[64:65, :], ln_sb[64:65, :],
                                         Exp, bias=0.0, scale=-1.0)
                    nc.scalar.activation(r_sb[32:33, :], ln_sb[32:33, :],
                                         Exp, bias=0.0, scale=-1.0)
                else:
                    nc.vector.reciprocal(r_sb[64:65, :], ota[64:65, :])
                    nc.vector.reciprocal(r_sb[32:33, :], otb[32:33, :])
                rep = repp.tile([P, SBW], f32)
                nc.gpsimd.dma_start(
                    out=rep[0:64, :], in_=_replicate_row_ap(r_sb[64:65, :]))
                nc.gpsimd.dma_start(
                    out=rep[64:128, :], in_=_replicate_row_ap(r_sb[32:33, :]))
                jblk = slice(j * SBW, (j + 1) * SBW)
                nc.vector.tensor_mul(
                    oc[0:64, jblk], ota[0:64, :], rep[0:64, :])
                nc.vector.tensor_mul(
                    oc[64:128, jblk], otb[64:128, :], rep[64:128, :])
                tc.cur_priority = _pri0

    # ---------------- Phase 3: output projection (partial) ----------------
    with tc.tile_pool(name="ysb", bufs=3) as ysbp, \
         tc.tile_pool(name="psy", bufs=4, space="PSUM") as psy:
        for st_ in range(T // P):
            y_sb = ysbp.tile([P, E], f32)
            for half in range(2):
                ps = psy.tile([P, SBW], f32)
                for c in range(NP):
                    nc.tensor.matmul(
                        ps,
                        lhsT=OT[c][:, st_ * P:(st_ + 1) * P],
                        rhs=wps[:, c, half * SBW:(half + 1) * SBW],
                        start=(c == 0), stop=(c == NP - 1),
                    )
                nc.vector.tensor_copy(y_sb[:, half * SBW:(half + 1) * SBW], ps)
            nc.sync.dma_start(out=y_d[st_ * P:(st_ + 1) * P, :], in_=y_sb)


def build_program(fast=True):
    DT = f32r if fast else f32
    nc = bass.Bass("TRN2", target_bir_lowering=False, debug=False)
    xT_d = nc.declare_dram_parameter("xT", [E, T], DT, isOutput=False).ap()
    wq_d = nc.declare_dram_parameter("wq", [E, NP * P], DT, isOutput=False).ap()
    wk_d = nc.declare_dram_parameter("wk", [E, NP * P], DT, isOutput=False).ap()
    wv_d = nc.declare_dram_parameter("wv", [E, NP * P], DT, isOutput=False).ap()
    wpT_d = nc.declare_dram_parameter("wpT", [NP * P, E], DT, isOutput=False).ap()
    tri_d = nc.declare_dram_parameter("tri", [P, P], DT, isOutput=False).ap()
    y_d = nc.declare_dram_parameter("y", [T, E], f32, isOutput=True).ap()

    with tile.TileContext(nc, pool_alloc_mode="queue") as tc:
        with ExitStack() as ctx:
            _build_body(nc, tc, ctx, DT, xT_d, wq_d, wk_d, wv_d, wpT_d,
                        tri_d, y_d)
    _split_excess_waits(nc)
    return nc


def make_tri():
    tt = np.arange(P)[:, None]
    ss = np.arange(P)[None, :]
    return (tt <= ss).astype(np.float32)


def make_in_maps(x, Wq, Wk, Wv, Wp):
    tri = make_tri()
    in_maps = []
    for b in range(B):
        for g in range(2):
            hs = slice(g * 8, g * 8 + 8)
            in_maps.append({
                "xT": np.ascontiguousarray(x[b].T),
                "wq": np.ascontiguousarray(
                    Wq[hs].transpose(1, 0, 2).reshape(E, 512)),
                "wk": np.ascontiguousarray(
                    Wk[hs].transpose(1, 0, 2).reshape(E, 512)),
                "wv": np.ascontiguousarray(
                    Wv[hs].transpose(1, 0, 2).reshape(E, 512)),
                "wpT": np.ascontiguousarray(Wp[:, g * 512:(g + 1) * 512].T),
                "tri": tri,
            })
    return in_maps


def kernel(x, Wq, Wk, Wv, Wp, bp):
    x = np.asarray(x, dtype=np.float32)
    Wq = np.asarray(Wq, dtype=np.float32)
    Wk = np.asarray(Wk, dtype=np.float32)
    Wv = np.asarray(Wv, dtype=np.float32)
    Wp = np.asarray(Wp, dtype=np.float32)
    bp = np.asarray(bp, dtype=np.float32)

    fast = os.environ.get("BASS_MHA_PRECISE", "0") != "1"
    if fast not in _PROGS:
        _PROGS[fast] = build_program(fast=fast)
    nc = _PROGS[fast]

    in_maps = make_in_maps(x, Wq, Wk, Wv, Wp)
    res = run_bass_kernel_spmd(nc, in_maps, list(range(NCORES)))
    LAST["res"] = res
    LAST["exec_time_ns"] = res.exec_time_ns

    ys = [res.results[i]["y"] for i in range(NCORES)]
    out = np.stack([ys[2 * b] + ys[2 * b + 1] for b in range(B)], axis=0)
    out += bp[None, None, :]
    return out.astype(np.float32)


# revision 31
# speedup vs baseline: 1.0147x; 1.0147x over previous
"""Multi-head causal attention (B=4, T=2048, E=1024, H=16) on 8 NeuronCores.

Sharding: core = (batch b, head-group g of 8 heads). Each core computes its
heads' attention + a partial output projection; host sums the two partials
per batch and adds the bias (the "all-reduce" of the tensor-parallel plan).

Device layout (per core):
  xT  [E, T]   x[b] transposed on host; e on partitions (8 chunks of 128).
  wq/wk/wv [E, 512]: this core's 8 heads' weights, columns = h_local*64+d.
  wpT [512, E]: Wp rows for this core's 512 e-dims, transposed on host.
  QT/KT pair tiles [(2 heads' d)=128, T]; V tiles [t, 193] carry both heads'
  values plus ones/zeros columns so the PV matmuls emit softmax denominators
  into spare PSUM rows:
    cols 0:64    V_h0   -> ota rows 0:64   = OT_h0
    col  64      ones   -> ota row 64      = l_h0
    cols 65:97,98:129 zeros -> otb rows 0:31,33:63 = 0
    col  97      ones   -> otb row 32      = l_h1
    cols 129:193 V_h1   -> otb rows 64:128 = OT_h1 (lands on right partitions)
  Scores are computed transposed ST[t, s] (two heads' K=64 matmuls), exp on
  ACT with the 1/8 scale folded in, causality via reduced-width matmuls plus
  one [128,128] triangle mask on the diagonal subtile. Reciprocal runs on 2
  partitions (cost scales with free-size, not lanes), then a step-0 free-dim
  DMA replicates it across partitions for the normalize multiply. The
  normalized OT[(h,d), s] is exactly the lhsT the output projection needs.

FAST mode (default) tags matmul operands float32r (tf32-like, 11-bit
mantissa, 1 PE cycle/row); PRECISE mode keeps everything fp32 (4 cycles/row).
Set BASS_MHA_PRECISE=1 to force the exact variant.
"""

import os
import sys
from contextlib import ExitStack

import numpy as np

sys.path.insert(0, "/opt/trn_rl_repo")

import concourse.bass as bass
import concourse.tile as tile
from concourse import mybir
from concourse.bass_utils import run_bass_kernel_spmd

f32 = mybir.dt.float32
f32r = mybir.dt.float32r

B, T, E, H = 4, 2048, 1024, 16
DH = E // H          # 64
P = 128              # partitions
EC = E // P          # 8 e-chunks
NP = 4               # head pairs per core (8 heads)
SBW = 512            # s-block width
SB = T // SBW        # 4 s-blocks
TT = T // P          # 16 t-tiles
VW = 193             # V tile width: V0|1|zeros|1|V1
NCORES = 8

_PROGS = {}
LAST = {}


def _split_excess_waits(nc, limit=1):
    """walrus in this container encodes at most one sync-wait per
    instruction; move extras onto same-engine NOPs placed just before."""
    for fn in nc.m.functions:
        for bb in fn.blocks:
            out = []
            changed = False
            for inst in bb.instructions:
                si = inst.sync_info
                if si is not None and si.on_wait and len(si.on_wait) > limit:
                    waits = list(si.on_wait)
                    extra, keep = waits[:-limit], waits[-limit:]
                    for k, w in enumerate(extra):
                        nop = mybir.InstNoOp(
                            name=f"{inst.name}-wsplit{k}", ins=[], outs=[]
                        )
                        nop.engine = inst.engine
                        nop.sync_info = type(si)(on_wait=[w], on_update=[])
                        nc.register_instruction(nop)
                        out.append(nop)
                    si.on_wait = keep
                    changed = True
                out.append(inst)
            if changed:
                bb.instructions = out
    return nc


def _replicate_row_ap(src_row):
    """AP reading one SBUF partition row [1, W] as [1, 64, W] via a step-0
    free dim — DMA'd to a [64, W] destination this replicates the row."""
    return bass.AP(
        tensor=src_row.tensor,
        offset=src_row.offset,
        ap=[list(src_row.ap[0]), [0, 64], list(src_row.ap[1])],
    )


def _build_body(nc, tc, ctx, DT, xT_d, wq_d, wk_d, wv_d, wpT_d, tri_d, y_d):
    Exp = mybir.ActivationFunctionType.Exp
    Ln = mybir.ActivationFunctionType.Ln

    persist = ctx.enter_context(tc.tile_pool(name="persist", bufs=1))
    QT = [persist.tile([P, T], DT, tag=f"qt{p}", name=f"qt{p}") for p in range(NP)]
    V = [persist.tile([P, TT, VW], DT, tag=f"v{p}", name=f"v{p}") for p in range(NP)]
    tri_sb = persist.tile([P, P], DT, tag="tri")
    nc.sync.dma_start(out=tri_sb, in_=tri_d)

    # K staging in DRAM, already in the zero-padded KTA/KTB layout
    kta_dram = nc.dram_tensor("kta_dram", [NP, P, T], DT).ap()
    ktb_dram = nc.dram_tensor("ktb_dram", [NP, P, T], DT).ap()
    zero_sb = persist.tile([P, SBW], DT, tag="zeros")
    nc.vector.memset(zero_sb.bitcast(f32), 0.0)
    for p_ in range(NP):
        for mb in range(SB):
            mblk = slice(mb * SBW, (mb + 1) * SBW)
            nc.gpsimd.dma_start(
                out=kta_dram[p_][64:128, mblk], in_=zero_sb[0:64, :])
            nc.gpsimd.dma_start(
                out=ktb_dram[p_][0:64, mblk], in_=zero_sb[0:64, :])

    # ---------------- Phase 1: QKV projections ----------------
    with tc.tile_pool(name="xp", bufs=1) as xp:
        xTs = xp.tile([P, EC, T], DT)
        nc.sync.dma_start(out=xTs[:, 0, :], in_=xT_d[0:P, :])

        _ps1_ctx = ExitStack()
        ps1 = _ps1_ctx.enter_context(tc.tile_pool(name="ps1", bufs=4, space="PSUM"))

        wpool = ctx_w = ExitStack()
        wkp = ctx_w.enter_context(tc.tile_pool(name="wk", bufs=1))
        wqp = ctx_w.enter_context(tc.tile_pool(name="wq", bufs=1))
        wvp = ctx_w.enter_context(tc.tile_pool(name="wv", bufs=1))
        wks = wkp.tile([P, EC, NP * P], DT)
        wqs = wqp.tile([P, EC, NP * P], DT)
        wvs = wvp.tile([P, EC, NP * P], DT)
        wv_r = wv_d.rearrange("(c p) m -> p c m", p=P)
        wq_r = wq_d.rearrange("(c p) m -> p c m", p=P)
        wk_r = wk_d.rearrange("(c p) m -> p c m", p=P)
        # issue in matmul consumption order: (wv_c, xT_c) pairs, then wq, wk
        nc.sync.dma_start(out=wvs[:, 0, :], in_=wv_r[:, 0, :])
        for c in range(1, EC):
            nc.sync.dma_start(out=wvs[:, c, :], in_=wv_r[:, c, :])
            nc.sync.dma_start(out=xTs[:, c, :], in_=xT_d[c * P:(c + 1) * P, :])
        for c in range(EC):
            nc.sync.dma_start(out=wqs[:, c, :], in_=wq_r[:, c, :])
        for c in range(EC):
            nc.sync.dma_start(out=wks[:, c, :], in_=wk_r[:, c, :])

        # V natural for all 4 pairs at once (N=512), scattered into V tiles
        if True:
            for p_ in range(NP):
                nc.vector.memset(V[p_][:, :, 64:65].bitcast(f32), 1.0)
                nc.vector.memset(V[p_][:, :, 65:97].bitcast(f32), 0.0)
                nc.vector.memset(V[p_][:, :, 97:98].bitcast(f32), 1.0)
                nc.vector.memset(V[p_][:, :, 98:129].bitcast(f32), 0.0)
            for tt in range(TT):
                ps = ps1.tile([P, NP * P], f32, tag="ps1", name="ps")
                for c in range(EC):
                    nc.tensor.matmul(
                        ps,
                        lhsT=xTs[:, c, tt * P:(tt + 1) * P],
                        rhs=wvs[:, c, :],
                        start=(c == 0), stop=(c == EC - 1),
                    )
                for p_ in range(NP):
                    nc.vector.tensor_copy(
                        V[p_][:, tt, 0:64], ps[:, p_ * P:p_ * P + 64])
                    nc.vector.tensor_copy(
                        V[p_][:, tt, 129:193], ps[:, p_ * P + 64:(p_ + 1) * P])

        # QT pairs [(2 heads' d), s]
        if True:
            for p_ in range(NP):
                for m in range(SB):
                    ps = ps1.tile([P, SBW], f32, tag="ps1", name="ps")
                    for c in range(EC):
                        nc.tensor.matmul(
                            ps,
                            lhsT=wqs[:, c, p_ * P:(p_ + 1) * P],
                            rhs=xTs[:, c, m * SBW:(m + 1) * SBW],
                            start=(c == 0), stop=(c == EC - 1),
                        )
                    nc.vector.tensor_copy(QT[p_][:, m * SBW:(m + 1) * SBW], ps)

        # KT pairs [(2 heads' d), t] -> straight to DRAM staging via bounce
        with tc.tile_pool(name="bounce", bufs=3) as bncp:
            for p_ in range(NP):
                for m in range(SB):
                    ps = ps1.tile([P, SBW], f32, tag="ps1", name="ps")
                    for c in range(EC):
                        nc.tensor.matmul(
                            ps,
                            lhsT=wks[:, c, p_ * P:(p_ + 1) * P],
                            rhs=xTs[:, c, m * SBW:(m + 1) * SBW],
                            start=(c == 0), stop=(c == EC - 1),
                        )
                    bnc = bncp.tile([P, SBW], DT)
                    nc.vector.tensor_copy(bnc, ps)
                    mblk = slice(m * SBW, (m + 1) * SBW)
                    nc.gpsimd.dma_start(
                        out=kta_dram[p_][0:64, mblk], in_=bnc[0:64, :])
                    nc.gpsimd.dma_start(
                        out=ktb_dram[p_][64:128, mblk], in_=bnc[64:128, :])
        ctx_w.close()

    _ps1_ctx.close()

    # ---------------- Phase 2: causal attention ----------------
    # zero-padded K=128 score operands (rolling, rebuilt per pair):
    # KTA = [KT_h0; 0], KTB = [0; KT_h1]
    ktab_pool = ctx.enter_context(tc.tile_pool(name="ktab", bufs=2))

    otp = ctx.enter_context(tc.tile_pool(name="otp", bufs=1))
    OT = [otp.tile([P, T], DT, tag=f"ot{p}", name=f"ot{p}") for p in range(NP)]
    with tc.tile_pool(name="pt", bufs=4) as ptp, \
         tc.tile_pool(name="lsb", bufs=2) as lsp, \
         tc.tile_pool(name="rsb", bufs=2) as rsp, \
         tc.tile_pool(name="rep", bufs=2) as repp, \
         tc.tile_pool(name="psst", bufs=2, space="PSUM") as psst, \
         tc.tile_pool(name="psota", bufs=2, space="PSUM") as psota, \
         tc.tile_pool(name="psotb", bufs=2, space="PSUM") as psotb:
        for p_ in range(NP):
            qt, vt, oc = QT[p_], V[p_], OT[p_]
            kta = ktab_pool.tile([P, T], DT, tag="kta", name="kta")
            ktb = ktab_pool.tile([P, T], DT, tag="ktb", name="ktb")
            nc.sync.dma_start(out=kta, in_=kta_dram[p_])
            nc.sync.dma_start(out=ktb, in_=ktb_dram[p_])
            for j in range(SB):
                ntt = 4 * (j + 1)
                ota = psota.tile([P, SBW], f32)
                otb = psotb.tile([P, SBW], f32)
                def _pv(pv_args):
                    pt_, s_lo_, i_ = pv_args
                    nc.tensor.matmul(
                        ota[:, s_lo_:SBW],
                        lhsT=vt[:, i_, 0:P],
                        rhs=pt_[:, s_lo_:SBW],
                        start=(i_ == 0), stop=(i_ == ntt - 1),
                    )
                    nc.tensor.matmul(
                        otb[:, s_lo_:SBW],
                        lhsT=vt[:, i_, 65:VW],
                        rhs=pt_[:, SBW + s_lo_:2 * SBW],
                        start=(i_ == 0), stop=(i_ == ntt - 1),
                    )

                pv_pending = None
                for i in range(ntt):
                    dd = i - 4 * j
                    s_lo = P * dd if dd >= 0 else 0
                    st = psst.tile([P, 2 * SBW], f32)
                    nc.tensor.matmul(
                        st[:, s_lo:SBW],
                        lhsT=kta[:, i * P:(i + 1) * P],
                        rhs=qt[:, j * SBW + s_lo:(j + 1) * SBW],
                        start=True, stop=True,
                    )
                    nc.tensor.matmul(
                        st[:, SBW + s_lo:2 * SBW],
                        lhsT=ktb[:, i * P:(i + 1) * P],
                        rhs=qt[:, j * SBW + s_lo:(j + 1) * SBW],
                        start=True, stop=True,
                    )
                    pt = ptp.tile([P, 2 * SBW], DT)
                    st3 = st.rearrange("p (h w) -> p h w", h=2)[:, :, s_lo:SBW]
                    pt3 = pt.rearrange("p (h w) -> p h w", h=2)[:, :, s_lo:SBW]
                    nc.scalar.activation(pt3, st3, Exp, bias=0.0, scale=0.125)
                    if dd >= 0:
                        nc.vector.tensor_mul(
                            pt[:, s_lo:s_lo + P], pt[:, s_lo:s_lo + P], tri_sb)
                        nc.vector.tensor_mul(
                            pt[:, SBW + s_lo:SBW + s_lo + P],
                            pt[:, SBW + s_lo:SBW + s_lo + P], tri_sb)
                    # software pipeline: PV for the previous iter issues after
                    # this iter's scores, so PE never waits on this iter's exp
                    if pv_pending is not None:
                        _pv(pv_pending)
                    pv_pending = (pt, s_lo, i)
                _pv(pv_pending)
                # finalize: l_h0 = ota row 64, l_h1 = otb row 32
                # deprioritized so it fills ACT/DVE idle slots instead of
                # bubbling the next block's score->exp->PV pipeline
                _pri0 = tc.cur_priority
                tc.cur_priority = _pri0 + 16
                r_sb = rsp.tile([P, SBW], f32)
                if DT is f32r:
                    # 1/l = exp(-ln(l)) on ACT (LUT err ~1e-5 << f32r rounding)
                    ln_sb = lsp.tile([P, SBW], f32)
                    nc.scalar.activation(ln_sb[64:65, :], ota[64:65, :], Ln)
                    nc.scalar.activation(ln_sb[32:33, :], otb[32:33, :], Ln)
                    nc.scalar.activation(r_sb[64:65, :], ln_sb[64:65, :],
                                         Exp, bias=0.0, scale=-1.0)
                    nc.scalar.activation(r_sb[32:33, :], ln_sb[32:33, :],
                                         Exp, bias=0.0, scale=-1.0)
                else:
                    nc.vector.reciprocal(r_sb[64:65, :], ota[64:65, :])
                    nc.vector.reciprocal(r_sb[32:33, :], otb[32:33, :])
                rep = repp.tile([P, SBW], f32)
                nc.gpsimd.dma_start(
                    out=rep[0:64, :], in_=_replicate_row_ap(r_sb[64:65, :]))
                nc.gpsimd.dma_start(
                    out=rep[64:128, :], in_=_replicate_row_ap(r_sb[32:33, :]))
                jblk = slice(j * SBW, (j + 1) * SBW)
                nc.vector.tensor_mul(
                    oc[0:64, jblk], ota[0:64, :], rep[0:64, :])
                nc.vector.tensor_mul(
                    oc[64:128, jblk], otb[64:128, :], rep[64:128, :])
                tc.cur_priority = _pri0

    # ---------------- Phase 3: output projection (partial) ----------------
    with tc.tile_pool(name="wp", bufs=1) as wpp, \
         tc.tile_pool(name="ysb", bufs=3) as ysbp, \
         tc.tile_pool(name="psy", bufs=4, space="PSUM") as psy:
        wps = wpp.tile([P, NP, E], DT)
        for c in range(NP):
            nc.sync.dma_start(
                out=wps[:, c, :],
                in_=wpT_d.rearrange("(c p) m -> p c m", p=P)[:, c, :])
        for st_ in range(T // P):
            y_sb = ysbp.tile([P, E], f32)
            for half in range(2):
                ps = psy.tile([P, SBW], f32)
                for c in range(NP):
                    nc.tensor.matmul(
                        ps,
                        lhsT=OT[c][:, st_ * P:(st_ + 1) * P],
                        rhs=wps[:, c, half * SBW:(half + 1) * SBW],
                        start=(c == 0), stop=(c == NP - 1),
                    )
                nc.vector.tensor_copy(y_sb[:, half * SBW:(half + 1) * SBW], ps)
            nc.sync.dma_start(out=y_d[st_ * P:(st_ + 1) * P, :], in_=y_sb)


def build_program(fast=True):
    DT = f32r if fast else f32
    nc = bass.Bass("TRN2", target_bir_lowering=False, debug=False)
    xT_d = nc.declare_dram_parameter("xT", [E, T], DT, isOutput=False).ap()
    wq_d = nc.declare_dram_parameter("wq", [E, NP * P], DT, isOutput=False).ap()
    wk_d = nc.declare_dram_parameter("wk", [E, NP * P], DT, isOutput=False).ap()
    wv_d = nc.declare_dram_parameter("wv", [E, NP * P], DT, isOutput=False).ap()
    wpT_d = nc.declare_dram_parameter("wpT", [NP * P, E], DT, isOutput=False).ap()
    tri_d = nc.declare_dram_parameter("tri", [P, P], DT, isOutput=False).ap()
    y_d = nc.declare_dram_parameter("y", [T, E], f32, isOutput=True).ap()

    with tile.TileContext(nc, pool_alloc_mode="queue") as tc:
        with ExitStack() as ctx:
            _build_body(nc, tc, ctx, DT, xT_d, wq_d, wk_d, wv_d, wpT_d,
                        tri_d, y_d)
    _split_excess_waits(nc)
    return nc


def make_tri():
    tt = np.arange(P)[:, None]
    ss = np.arange(P)[None, :]
    return (tt <= ss).astype(np.float32)


def make_in_maps(x, Wq, Wk, Wv, Wp):
    tri = make_tri()
    in_maps = []
    for b in range(B):
        for g in range(2):
            hs = slice(g * 8, g * 8 + 8)
            in_maps.append({
                "xT": np.ascontiguousarray(x[b].T),
                "wq": np.ascontiguousarray(
                    Wq[hs].transpose(1, 0, 2).reshape(E, 512)),
                "wk": np.ascontiguousarray(
                    Wk[hs].transpose(1, 0, 2).reshape(E, 512)),
                "wv": np.ascontiguousarray(
                    Wv[hs].transpose(1, 0, 2).reshape(E, 512)),
                "wpT": np.ascontiguousarray(Wp[:, g * 512:(g + 1) * 512].T),
                "tri": tri,
            })
    return in_maps


def kernel(x, Wq, Wk, Wv, Wp, bp):
    x = np.asarray(x, dtype=np.float32)
    Wq = np.asarray(Wq, dtype=np.float32)
    Wk = np.asarray(Wk, dtype=np.float32)
    Wv = np.asarray(Wv, dtype=np.float32)
    Wp = np.asarray(Wp, dtype=np.float32)
    bp = np.asarray(bp, dtype=np.float32)

    fast = os.environ.get("BASS_MHA_PRECISE", "0") != "1"
    if fast not in _PROGS:
        _PROGS[fast] = build_program(fast=fast)
    nc = _PROGS[fast]

    in_maps = make_in_maps(x, Wq, Wk, Wv, Wp)
    res = run_bass_kernel_spmd(nc, in_maps, list(range(NCORES)))
    LAST["res"] = res
    LAST["exec_time_ns"] = res.exec_time_ns

    ys = [res.results[i]["y"] for i in range(NCORES)]
    out = np.stack([ys[2 * b] + ys[2 * b + 1] for b in range(B)], axis=0)
    out += bp[None, None, :]
    return out.astype(np.float32)
